# revision 19
# baseline (speedup 1.0000x reference)
"""Multi-head attention block (B=8, N=1024, H=8, d=128, D_in=256) on 8 trn2 cores.

Sharding: data-parallel over batch — core b computes batch element b entirely
(8 heads), no collectives. Host pre-transposes x, pre-scales wq by 1/sqrt(d),
and ships the additive [N,N] bias as exp(B)^T so the device computes
exp(S+B) = exp(S) * expB.

bf16 pipeline, HEAD-granular pipeline with stationary-weight sharing:
  * TRN2 serializes each matmul behind its ~90ns stationary load; a matmul
    with `ldweights=False` reuses the previously loaded stationary
    (measured -73ns/matmul). The loop processes both n-halves of a head
    together so S (shared KT chunk), PV (shared V slice) and the QKV
    setup matmuls (shared weight chunk) run as no-reload pairs.
  * S matmuls for the two n-halves of m-chunk m land in one 2-bank PSUM
    group [128, 2(half), 512]; ONE ACT exp per m (full row), ONE DVE mul
    by the exp(B)^T row (bf16 16-bit 2x mode).
  * softmax denominators: per-m ones matmuls accumulate into a shared
    PSUM bank (half 0 at partition 0, half 1 at partition 32); DVE
    approx-reciprocal per half; GPSIMD partition_broadcast expands
    rc[1,512] -> bc[128,512] SBUF (no PE broadcast matmul, no ACT copy).
  * K-bias dropped (cancels in softmax); V-bias folded into proj_b on the
    host; Q-bias rides the ACT PSUM->SBUF copy.
  (fp8 DoubleRow was tried and reverted: real-TRN2 DR matmuls stream at
  the same rows/cycle as bf16, unlike the cost model.)

PSUM banks: S pool 2 groups x 2 banks (also serves setup pieces), pv 2,
rs 1 (both halves), pj 1.

Per-core dataflow (all matmuls bf16, moving free dim 512):
  QT[c][128,2,512], KT, V via setup pieces (4 matmuls + 1 chunk copy each)
  head loop u (software-pipelined 2 deep), m-chunk loop 0..7:
    S pair (shared KT stationary) -> exp -> at[128,2,512] *= expB row
    rs pair (ones) + pv pair (shared V stationary) consume at[:, i, :]
  2x recip + gpsimd bcast per head; oh = pv * bc; pj = pw_h.T @ oh;
  yacc += pj;  yT = yacc + proj_b' -> DRAM [128, 1024]; host transposes.
"""

import math
import sys

import numpy as np

if "/opt/trn_rl_repo" not in sys.path:
    sys.path.insert(0, "/opt/trn_rl_repo")

import ml_dtypes

import concourse.bass as bass
import concourse.bass_isa as bass_isa
import concourse.tile as tile
from concourse import bacc
from concourse import library_config
from concourse import mybir

F32 = mybir.dt.float32
BF16 = mybir.dt.bfloat16
EXP = mybir.ActivationFunctionType.Exp
IDENT = mybir.ActivationFunctionType.Identity

N = 1024          # sequence length
D_IN = 256        # input dim
H = 8             # heads
DH = 128          # head dim
C = H * DH        # 1024
NCORES = 8
HALF = 512        # matmul moving free dim
NM = 8            # m-chunks per head
SHARE_LDW = True  # reuse loaded stationaries (ldweights=False on 2nd of pair)


def build_nc():
    nc = bacc.Bacc("TRN2", target_bir_lowering=False, debug=False,
                   num_devices=NCORES)

    xt_d = nc.dram_tensor("xt", [128, 2 * N], BF16, kind="ExternalInput").ap()
    wqk0_d = nc.dram_tensor("wqk0", [128, 4 * 128], BF16,
                            kind="ExternalInput").ap()
    bias_d = nc.dram_tensor("biases", [128, 9], F32,
                            kind="ExternalInput").ap()
    wv_d = nc.dram_tensor("wv", [128, 2 * 1024], BF16,
                          kind="ExternalInput").ap()
    wbig_d = nc.dram_tensor("wbig", [128, 2 * 2304], BF16,
                            kind="ExternalInput").ap()
    eb_d = nc.dram_tensor("eb", [128, 8 * N], BF16, kind="ExternalInput").ap()
    yT = nc.dram_tensor("yT", [DH, N], F32, kind="ExternalOutput").ap()

    with tile.TileContext(nc) as tc:
        build_body(nc, tc, xt_d, wqk0_d, bias_d, wv_d, wbig_d, eb_d, yT)
    nc.compile()
    return nc


def build_body(nc, tc, xt_d, wqk0_d, bias_d, wv_d, wbig_d, eb_d, yT):
    with (
        tc.tile_pool(name="persist", bufs=1) as P,
        tc.tile_pool(name="at", bufs=10) as AT,
        tc.tile_pool(name="oh", bufs=3) as OH,
        tc.tile_pool(name="rc", bufs=3) as RC,
        tc.tile_pool(name="bc", bufs=3) as BC,
        tc.tile_pool(name="ps_s", bufs=2, space="PSUM") as PS_S,
        tc.tile_pool(name="ps_pv", bufs=2, space="PSUM") as PS_PV,
        tc.tile_pool(name="ps_rs", bufs=1, space="PSUM") as PS_RS,
        tc.tile_pool(name="ps_pj", bufs=1, space="PSUM") as PS_PJ,
    ):
        # gpsimd library for partition_broadcast; loads while inputs DMA
        nc.gpsimd.load_library(library_config.attn)

        # ---- input DMAs: critical-path first
        xt_all = [P.tile([128, N], BF16, tag=f"xt{d}", name=f"xt{d}")
                  for d in range(2)]
        for d in range(2):
            nc.sync.dma_start(out=xt_all[d], in_=xt_d[:, d * N:(d + 1) * N])
        wqk0 = P.tile([128, 2, 2, 128], BF16, tag="wqk0")
        nc.sync.dma_start(out=wqk0, in_=wqk0_d.rearrange(
            "p (w a c) -> p w a c", w=2, a=2))
        bias_all = P.tile([128, 9], F32, tag="bias")
        nc.sync.dma_start(out=bias_all, in_=bias_d)
        wv_sb = P.tile([128, 2, 1024], BF16, tag="wv")
        nc.sync.dma_start(out=wv_sb, in_=wv_d.rearrange("p (a c) -> p a c",
                                                        a=2))
        wbig = P.tile([128, 2, 2304], BF16, tag="wbig")
        nc.sync.dma_start(out=wbig, in_=wbig_d.rearrange("p (a c) -> p a c",
                                                         a=2))
        # eb rows as [128, 4, 2, 512] so a full n-row views as [128, 2, 512]
        eb_sb = [P.tile([128, 4, 2, HALF], BF16, tag=f"eb{h}", name=f"eb{h}")
                 for h in range(2)]
        for h in range(2):
            nc.sync.dma_start(out=eb_sb[h], in_=eb_d[:, h * 4 * N:(h + 1) * 4 * N]
                              .rearrange("p (a i n) -> p a i n", a=4, i=2))

        def eb_row(m):   # [128, 2, 512] view of expB for m-chunk m
            return eb_sb[m // 4][:, m % 4, :, :]

        wqb_sb = bias_all[:, 0:8]
        pb_sb = bias_all[:, 8:9]

        def pw_view(h):  # pw head h lives in the d=h//4 tail of wbig
            o = 1792 + (h % 4) * 128
            return wbig[:, h // 4, o:o + 128]

        # ---- persistent tiles ----
        ones = P.tile([128, 1], BF16, tag="ones")
        with tc.tile_pool(name="mkconst", bufs=1) as MK:
            ones_f = MK.tile([128, 1], F32, tag="ones_f")
            nc.vector.memset(ones_f, 1.0)
            nc.vector.tensor_copy(ones, ones_f)
            warm = MK.tile([128, 1], F32, tag="warm")
            nc.scalar.activation(warm, ones_f, func=EXP)
        qt_sb = [P.tile([128, 2, HALF], BF16, tag=f"qt{c}", name=f"qt{c}")
                 for c in range(8)]
        kt_sb = [P.tile([128, 2, HALF], BF16, tag=f"kt{c}", name=f"kt{c}")
                 for c in range(8)]
        v_sb = [P.tile([128, 2, HALF], BF16, tag=f"v{n}", name=f"v{n}")
                for n in range(8)]

        def kt_view(h, m):   # kt head h, m-chunk slice [128, 128]
            return kt_sb[h][:, m // 4, (m % 4) * 128:(m % 4 + 1) * 128]

        def v_view(m, h):    # v m-chunk, head-dim slice [128, 128]
            return v_sb[m][:, h // 4, (h % 4) * 128:(h % 4 + 1) * 128]

        yacc = P.tile([128, N], F32, tag="yacc")
        yt_sb = P.tile([128, N], F32, tag="yt")

        # ---- setup pieces: d-major matmul order shares each stationary
        def qkt_piece(wname, dst, c):
            wi = 0 if wname == "wq" else 1
            g = PS_S.tile([128, 2, HALF], F32, tag="sg")
            for d in range(2):
                if c == 0:
                    wt = wqk0[:, wi, d, :]
                else:
                    wt = wbig[:, d, wi * 896 + (c - 1) * 128:
                              wi * 896 + c * 128]
                for i in range(2):
                    mm = nc.tensor.matmul(
                        g[:, i, :], wt, xt_all[d][:, i * HALF:(i + 1) * HALF],
                        start=(d == 0), stop=(d == 1))
                    if i == 1 and SHARE_LDW:
                        mm.ins.ldweights = False
            if wname == "wq":
                nc.scalar.activation(dst[c], g, func=IDENT,
                                     bias=wqb_sb[:, c:c + 1])
            else:
                nc.vector.tensor_copy(dst[c], g)

        def v_piece(n):
            nsl = slice(n * 128, (n + 1) * 128)
            g = PS_S.tile([128, 2, HALF], F32, tag="sg")
            for d in range(2):
                for i in range(2):
                    mm = nc.tensor.matmul(
                        g[:, i, :], xt_all[d][:, nsl],
                        wv_sb[:, d, i * HALF:(i + 1) * HALF],
                        start=(d == 0), stop=(d == 1))
                    if i == 1 and SHARE_LDW:
                        mm.ins.ldweights = False
            nc.vector.tensor_copy(v_sb[n], g)

        qkt_piece("wq", qt_sb, 0)
        qkt_piece("wk", kt_sb, 0)

        pieces = [lambda n=n: v_piece(n) for n in range(8)]
        for c in range(1, 8):
            pieces.append(lambda c=c: qkt_piece("wq", qt_sb, c))
            pieces.append(lambda c=c: qkt_piece("wk", kt_sb, c))
        # chunk c needed at head-iteration c; v needed at u=1
        piece_quota = {0: 10, 1: 2, 2: 2, 3: 2, 4: 2, 5: 2, 6: 2}

        # ---- pipelined head loop ----
        at_t = {}     # (h, m) -> at tile [128, 2(half), 512]
        pv_t = {}     # (h, i) -> pv psum tile
        rs_t = {}     # h -> rowsum psum bank (half 0 @ p0, half 1 @ p32)
        rc_t = {}     # (h, i) -> reciprocal rowsum [1, HALF]
        bc_t = {}     # (h, i) -> broadcast recip [128, HALF] SBUF
        oh_t = {}     # (h, i) -> normalized head-output tile

        def s_exp(h, m):
            g = PS_S.tile([128, 2, HALF], F32, tag="sg", name=f"sg{h}_{m}")
            kt = kt_view(h, m)
            for i in range(2):
                mm = nc.tensor.matmul(g[:, i, :], kt, qt_sb[h][:, i, :],
                                      start=True, stop=True)
                if i == 1 and SHARE_LDW:
                    mm.ins.ldweights = False
            at = AT.tile([128, 2, HALF], BF16, tag="at", name=f"at{h}_{m}")
            nc.scalar.activation(at, g, func=EXP)
            nc.vector.tensor_mul(at, at, eb_row(m))
            at_t[(h, m)] = at

        def ones_pv(h, m):
            if m == 0:
                rs_t[h] = PS_RS.tile([128, HALF], F32, tag="rs", name=f"rs{h}")
                for i in range(2):
                    pv_t[(h, i)] = PS_PV.tile([128, HALF], F32, tag="pv",
                                              name=f"pv{h}_{i}")
            at = at_t.pop((h, m))
            rs = rs_t[h]
            for i in range(2):
                mm = nc.tensor.matmul(rs[32 * i:32 * i + 1, :], ones,
                                      at[:, i, :], start=(m == 0),
                                      stop=(m == NM - 1))
                if i == 1 and SHARE_LDW:
                    mm.ins.ldweights = False
            vv = v_view(m, h)
            for i in range(2):
                mm = nc.tensor.matmul(pv_t[(h, i)], vv, at[:, i, :],
                                      start=(m == 0), stop=(m == NM - 1))
                if i == 1 and SHARE_LDW:
                    mm.ins.ldweights = False

        from concourse.dve_ops import (
            RECIP_APPROX_FAST_CONSTS,
            RECIPROCAL_APPROX_FAST,
        )

        def recip_bcast(h):
            rs = rs_t.pop(h)
            cc = RECIP_APPROX_FAST_CONSTS
            # one recip over partitions 0..32 (base-0 AP: the custom DVE op
            # mis-reads non-zero partition offsets); rows 1..31 are unused
            rc = RC.tile([33, HALF], F32, tag="rc", name=f"rc{h}")
            nc.vector._custom_dve(RECIPROCAL_APPROX_FAST, out=rc,
                                  in0=rs[0:33, :], s0=cc["s0"], s1=cc["s1"],
                                  imm2=cc["imm2"])
            # half 1's row must be re-staged at partition base 0 for gpsimd
            rc1 = RC.tile([1, HALF], F32, tag="rc1", name=f"rc1_{h}")
            nc.vector.tensor_copy(rc1, rc[32:33, :])
            for i, src_ap in ((0, rc[0:1, :]), (1, rc1)):
                bc = BC.tile([128, HALF], F32, tag="bc", name=f"bc{h}_{i}")
                nc.gpsimd.partition_broadcast(bc, src_ap, channels=128)
                bc_t[(h, i)] = bc
            rc_t[h] = rc

        def oh_mul(h, i):
            oh = OH.tile([128, HALF], BF16, tag="oh", name=f"oh{h}_{i}")
            nc.vector.tensor_mul(oh, pv_t.pop((h, i)), bc_t.pop((h, i)))
            rc_t.pop(h, None)
            oh_t[(h, i)] = oh

        def proj_acc(h, i):
            ns = slice(i * HALF, (i + 1) * HALF)
            pj = PS_PJ.tile([128, HALF], F32, tag="pj", name=f"pj{h}_{i}")
            nc.tensor.matmul(pj, pw_view(h), oh_t.pop((h, i)),
                             start=True, stop=True)
            if h == 0:
                nc.vector.tensor_copy(yacc[:, ns], pj)
            else:
                nc.vector.tensor_add(yacc[:, ns], yacc[:, ns], pj)

        def finalize(i):
            ns = slice(i * HALF, (i + 1) * HALF)
            nc.scalar.activation(yt_sb[:, ns], yacc[:, ns], func=IDENT,
                                 bias=pb_sb)
            nc.sync.dma_start(out=yT[:, ns], in_=yt_sb[:, ns])

        pi = 0
        for u in range(H + 2):
            quota = piece_quota.get(u, 0)
            for m in range(NM):
                if m == 0 and u >= 2:
                    oh_mul(u - 2, 0)   # frees pv buffers before reuse
                    oh_mul(u - 2, 1)
                if u < H:
                    s_exp(u, m)
                if 1 <= u <= H and m >= 1:
                    ones_pv(u - 1, m - 1)   # one slot late: S work covers
                                            # the pv-buffer wait on oh(u-2)
                if m == 2 and u >= 2:
                    proj_acc(u - 2, 0)
                if m == 5 and u >= 2:
                    proj_acc(u - 2, 1)
                if quota:
                    for _ in range(quota // NM + (1 if m < quota % NM else 0)):
                        pieces[pi](); pi += 1
            if 1 <= u <= H:
                ones_pv(u - 1, NM - 1)
                recip_bcast(u - 1)   # rs(u-1) just stopped
            if u == H + 1:
                finalize(0)
                finalize(1)
        assert pi == len(pieces), (pi, len(pieces))


_CACHE = {}


def _prep_inputs(x, B_bias, wq_w, wq_b, wk_w, wk_b, wv_w, wv_b, proj_w, proj_b):
    s = 1.0 / math.sqrt(DH)
    f = np.float32
    b16 = ml_dtypes.bfloat16

    def d2(w):  # [256, C] -> [2, 128, C]
        return np.asarray(w, f).reshape(2, 128, -1)

    wq3 = d2(np.asarray(wq_w) * s)
    wk3 = d2(wk_w)
    wv3 = d2(wv_w)
    # wqk0: [p, w, a, c0] packed
    wqk0 = np.stack([wq3[:, :, :128], wk3[:, :, :128]], 0)  # [w, a, p, 128]
    wqk0 = np.ascontiguousarray(
        wqk0.transpose(2, 0, 1, 3).reshape(128, -1).astype(b16))
    # wbig per d: [wq_r 896 | wk_r 896 | pw-half 512]
    pwf = np.asarray(proj_w, f).reshape(2, 512, DH)  # head-halves 0-3 / 4-7
    rows = []
    for d in range(2):
        pw_tail = pwf[d].reshape(4, 128, DH)
        pw_part = pw_tail.transpose(1, 0, 2).reshape(128, 512)
        rows.append(np.concatenate(
            [wq3[d, :, 128:], wk3[d, :, 128:], pw_part], axis=1))
    wbig = np.ascontiguousarray(np.stack(rows, 1).reshape(128, -1).astype(b16))
    wv_p = np.ascontiguousarray(
        wv3.transpose(1, 0, 2).reshape(128, -1).astype(b16))
    # biases: [wqb 8 | pb2 1]; k-bias dropped (cancels in softmax),
    # v-bias folded into the projection bias.
    wqb_t = (np.asarray(wq_b, f) * s).reshape(8, 128).T
    pb2 = (np.asarray(proj_b, f)
           + np.asarray(wv_b, f) @ np.asarray(proj_w, f)).reshape(128, 1)
    bias_all = np.ascontiguousarray(
        np.concatenate([wqb_t, pb2], axis=1).astype(f))
    # eb: exp(B)^T chunks packed [p, (m n)]
    ebh = np.exp(np.asarray(B_bias, np.float32).T).reshape(8, 128, N)
    eb_all = np.ascontiguousarray(
        ebh.transpose(1, 0, 2).reshape(128, 8 * N).astype(b16))
    xTh = np.asarray(x, f).transpose(0, 2, 1).reshape(8, 2, 128, N)
    shared = dict(wqk0=wqk0, wbig=wbig, wv=wv_p, biases=bias_all, eb=eb_all)
    return [dict(shared, xt=np.ascontiguousarray(
        xTh[b].transpose(1, 0, 2).reshape(128, 2 * N).astype(b16)))
        for b in range(NCORES)]


def kernel(**inputs):
    from concourse.bass_utils import run_bass_kernel_spmd

    if "nc" not in _CACHE:
        _CACHE["nc"] = build_nc()
    nc = _CACHE["nc"]
    in_maps = _prep_inputs(**inputs)
    res = run_bass_kernel_spmd(nc, in_maps, core_ids=list(range(NCORES)))
    out = np.stack([np.asarray(res.results[b]["yT"]).T for b in range(NCORES)])
    return np.ascontiguousarray(out.astype(np.float32))


# revision 20
# speedup vs baseline: 1.1674x; 1.1674x over previous
"""Multi-head attention block (B=8, N=1024, H=8, d=128, D_in=256) on 8 trn2 cores.

Sharding: data-parallel over batch — core b computes batch element b entirely
(8 heads), no collectives. Host pre-transposes x, pre-scales wq by 1/sqrt(d),
and ships the additive [N,N] bias as exp(B)^T so the device computes
exp(S+B) = exp(S) * expB.

bf16 pipeline, HEAD-granular pipeline with stationary-weight sharing:
  * TRN2 serializes each matmul behind its ~90ns stationary load; a matmul
    with `ldweights=False` reuses the previously loaded stationary
    (measured -73ns/matmul). The loop processes both n-halves of a head
    together so S (shared KT chunk), PV (shared V slice) and the QKV
    setup matmuls (shared weight chunk) run as no-reload pairs.
  * S matmuls for the two n-halves of m-chunk m land in one 2-bank PSUM
    group [128, 2(half), 512]; ONE ACT exp per m (full row), ONE DVE mul
    by the exp(B)^T row (bf16 16-bit 2x mode).
  * softmax denominators: per-m ones matmuls accumulate into a shared
    PSUM bank (half 0 at partition 0, half 1 at partition 32); DVE
    approx-reciprocal per half; GPSIMD partition_broadcast expands
    rc[1,512] -> bc[128,512] SBUF (no PE broadcast matmul, no ACT copy).
  * K-bias dropped (cancels in softmax); V-bias folded into proj_b on the
    host; Q-bias rides the ACT PSUM->SBUF copy.
  (fp8 DoubleRow was tried and reverted: real-TRN2 DR matmuls stream at
  the same rows/cycle as bf16, unlike the cost model.)

PSUM banks: S pool 2 groups x 2 banks (also serves setup pieces), pv 2,
rs 1 (both halves), pj 1.

Per-core dataflow (all matmuls bf16, moving free dim 512):
  QT[c][128,2,512], KT, V via setup pieces (4 matmuls + 1 chunk copy each)
  head loop u (software-pipelined 2 deep), m-chunk loop 0..7:
    S pair (shared KT stationary) -> exp -> at[128,2,512] *= expB row
    rs pair (ones) + pv pair (shared V stationary) consume at[:, i, :]
  2x recip + gpsimd bcast per head; oh = pv * bc; pj = pw_h.T @ oh;
  yacc += pj;  yT = yacc + proj_b' -> DRAM [128, 1024]; host transposes.
"""

import math
import sys

import numpy as np

if "/opt/trn_rl_repo" not in sys.path:
    sys.path.insert(0, "/opt/trn_rl_repo")

import ml_dtypes

import concourse.bass as bass
import concourse.bass_isa as bass_isa
import concourse.tile as tile
from concourse import bacc
from concourse import library_config
from concourse import mybir

F32 = mybir.dt.float32
BF16 = mybir.dt.bfloat16
EXP = mybir.ActivationFunctionType.Exp
IDENT = mybir.ActivationFunctionType.Identity

N = 1024          # sequence length
D_IN = 256        # input dim
H = 8             # heads
DH = 128          # head dim
C = H * DH        # 1024
NCORES = 8
HALF = 512        # matmul moving free dim
NM = 8            # m-chunks per head
SHARE_LDW = True  # reuse loaded stationaries (ldweights=False on 2nd of pair)


def build_nc():
    nc = bacc.Bacc("TRN2", target_bir_lowering=False, debug=False,
                   num_devices=NCORES)

    xt_d = nc.dram_tensor("xt", [128, 2 * N], BF16, kind="ExternalInput").ap()
    wqk0_d = nc.dram_tensor("wqk0", [128, 4 * 128], BF16,
                            kind="ExternalInput").ap()
    bias_d = nc.dram_tensor("biases", [128, 9], F32,
                            kind="ExternalInput").ap()
    wv_d = nc.dram_tensor("wv", [128, 2 * 1024], BF16,
                          kind="ExternalInput").ap()
    wbig_d = nc.dram_tensor("wbig", [128, 2 * 2304], BF16,
                            kind="ExternalInput").ap()
    eb_d = nc.dram_tensor("eb", [128, 8 * N], BF16, kind="ExternalInput").ap()
    yT = nc.dram_tensor("yT", [DH, N], F32, kind="ExternalOutput").ap()

    with tile.TileContext(nc) as tc:
        build_body(nc, tc, xt_d, wqk0_d, bias_d, wv_d, wbig_d, eb_d, yT)
    nc.compile()
    return nc


def build_body(nc, tc, xt_d, wqk0_d, bias_d, wv_d, wbig_d, eb_d, yT):
    with (
        tc.tile_pool(name="persist", bufs=1) as P,
        tc.tile_pool(name="at", bufs=10) as AT,
        tc.tile_pool(name="oh", bufs=3) as OH,
        tc.tile_pool(name="rc", bufs=3) as RC,
        tc.tile_pool(name="bc", bufs=3) as BC,
        tc.tile_pool(name="ps_s", bufs=2, space="PSUM") as PS_S,
        tc.tile_pool(name="ps_pv", bufs=2, space="PSUM") as PS_PV,
        tc.tile_pool(name="ps_rs", bufs=1, space="PSUM") as PS_RS,
        tc.tile_pool(name="ps_pj", bufs=1, space="PSUM") as PS_PJ,
    ):
        # gpsimd library for partition_broadcast; loads while inputs DMA
        nc.gpsimd.load_library(library_config.attn)

        # ---- input DMAs: critical-path first
        xt_all = [P.tile([128, N], BF16, tag=f"xt{d}", name=f"xt{d}")
                  for d in range(2)]
        for d in range(2):
            nc.sync.dma_start(out=xt_all[d], in_=xt_d[:, d * N:(d + 1) * N])
        wqk0 = P.tile([128, 2, 2, 128], BF16, tag="wqk0")
        nc.sync.dma_start(out=wqk0, in_=wqk0_d.rearrange(
            "p (w a c) -> p w a c", w=2, a=2))
        bias_all = P.tile([128, 9], F32, tag="bias")
        nc.sync.dma_start(out=bias_all, in_=bias_d)
        wv_sb = P.tile([128, 2, 1024], BF16, tag="wv")
        nc.sync.dma_start(out=wv_sb, in_=wv_d.rearrange("p (a c) -> p a c",
                                                        a=2))
        wbig = P.tile([128, 2, 2304], BF16, tag="wbig")
        nc.sync.dma_start(out=wbig, in_=wbig_d.rearrange("p (a c) -> p a c",
                                                         a=2))
        # eb rows as [128, 4, 2, 512] so a full n-row views as [128, 2, 512]
        eb_sb = [P.tile([128, 4, 2, HALF], BF16, tag=f"eb{h}", name=f"eb{h}")
                 for h in range(2)]
        for h in range(2):
            nc.sync.dma_start(out=eb_sb[h], in_=eb_d[:, h * 4 * N:(h + 1) * 4 * N]
                              .rearrange("p (a i n) -> p a i n", a=4, i=2))

        def eb_row(m):   # [128, 2, 512] view of expB for m-chunk m
            return eb_sb[m // 4][:, m % 4, :, :]

        wqb_sb = bias_all[:, 0:8]
        pb_sb = bias_all[:, 8:9]

        def pw_view(h):  # pw head h lives in the d=h//4 tail of wbig
            o = 1792 + (h % 4) * 128
            return wbig[:, h // 4, o:o + 128]

        # ---- persistent tiles ----
        ones = P.tile([128, 1], BF16, tag="ones")
        with tc.tile_pool(name="mkconst", bufs=1) as MK:
            ones_f = MK.tile([128, 1], F32, tag="ones_f")
            nc.vector.memset(ones_f, 1.0)
            nc.vector.tensor_copy(ones, ones_f)
            warm = MK.tile([128, 1], F32, tag="warm")
            nc.scalar.activation(warm, ones_f, func=EXP)
        qt_sb = [P.tile([128, 2, HALF], BF16, tag=f"qt{c}", name=f"qt{c}")
                 for c in range(8)]
        kt_sb = [P.tile([128, 2, HALF], BF16, tag=f"kt{c}", name=f"kt{c}")
                 for c in range(8)]
        v_sb = [P.tile([128, 2, HALF], BF16, tag=f"v{n}", name=f"v{n}")
                for n in range(8)]

        def kt_view(h, m):   # kt head h, m-chunk slice [128, 128]
            return kt_sb[h][:, m // 4, (m % 4) * 128:(m % 4 + 1) * 128]

        def v_view(m, h):    # v m-chunk, head-dim slice [128, 128]
            return v_sb[m][:, h // 4, (h % 4) * 128:(h % 4 + 1) * 128]

        yacc = P.tile([128, N], F32, tag="yacc")
        yt_sb = P.tile([128, N], F32, tag="yt")

        # ---- setup pieces: d-major matmul order shares each stationary
        def qkt_piece(wname, dst, c):
            wi = 0 if wname == "wq" else 1
            g = PS_S.tile([128, 2, HALF], F32, tag="sg")
            for d in range(2):
                if c == 0:
                    wt = wqk0[:, wi, d, :]
                else:
                    wt = wbig[:, d, wi * 896 + (c - 1) * 128:
                              wi * 896 + c * 128]
                for i in range(2):
                    mm = nc.tensor.matmul(
                        g[:, i, :], wt, xt_all[d][:, i * HALF:(i + 1) * HALF],
                        start=(d == 0), stop=(d == 1))
                    if i == 1 and SHARE_LDW:
                        mm.ins.ldweights = False
            if wname == "wq":
                nc.scalar.activation(dst[c], g, func=IDENT,
                                     bias=wqb_sb[:, c:c + 1])
            else:
                nc.vector.tensor_copy(dst[c], g)

        def v_piece(n):
            nsl = slice(n * 128, (n + 1) * 128)
            g = PS_S.tile([128, 2, HALF], F32, tag="sg")
            for d in range(2):
                for i in range(2):
                    mm = nc.tensor.matmul(
                        g[:, i, :], xt_all[d][:, nsl],
                        wv_sb[:, d, i * HALF:(i + 1) * HALF],
                        start=(d == 0), stop=(d == 1))
                    if i == 1 and SHARE_LDW:
                        mm.ins.ldweights = False
            nc.vector.tensor_copy(v_sb[n], g)

        qkt_piece("wq", qt_sb, 0)
        qkt_piece("wk", kt_sb, 0)

        pieces = [lambda n=n: v_piece(n) for n in range(8)]
        for c in range(1, 8):
            pieces.append(lambda c=c: qkt_piece("wq", qt_sb, c))
            pieces.append(lambda c=c: qkt_piece("wk", kt_sb, c))
        # chunk c needed at head-iteration c; v needed at u=1
        piece_quota = {0: 10, 1: 2, 2: 2, 3: 2, 4: 2, 5: 2, 6: 2}

        # ---- pipelined head loop ----
        at_t = {}     # (h, m) -> at tile [128, 2(half), 512]
        pv_t = {}     # (h, i) -> pv psum tile
        rs_t = {}     # h -> rowsum psum bank (half 0 @ p0, half 1 @ p32)
        rc_t = {}     # (h, i) -> reciprocal rowsum [1, HALF]
        bc_t = {}     # (h, i) -> broadcast recip [128, HALF] SBUF
        oh_t = {}     # (h, i) -> normalized head-output tile

        def s_exp(h, m):
            g = PS_S.tile([128, 2, HALF], F32, tag="sg", name=f"sg{h}_{m}")
            kt = kt_view(h, m)
            for i in range(2):
                mm = nc.tensor.matmul(g[:, i, :], kt, qt_sb[h][:, i, :],
                                      start=True, stop=True)
                if i == 1 and SHARE_LDW:
                    mm.ins.ldweights = False
            at = AT.tile([128, 2, HALF], BF16, tag="at", name=f"at{h}_{m}")
            nc.scalar.activation(at, g, func=EXP)
            nc.vector.tensor_mul(at, at, eb_row(m))
            at_t[(h, m)] = at

        def ones_pv(h, m):
            if m == 0:
                rs_t[h] = PS_RS.tile([128, HALF], F32, tag="rs", name=f"rs{h}")
                for i in range(2):
                    pv_t[(h, i)] = PS_PV.tile([128, HALF], F32, tag="pv",
                                              name=f"pv{h}_{i}")
            at = at_t.pop((h, m))
            rs = rs_t[h]
            for i in range(2):
                mm = nc.tensor.matmul(rs[32 * i:32 * i + 1, :], ones,
                                      at[:, i, :], start=(m == 0),
                                      stop=(m == NM - 1))
                if i == 1 and SHARE_LDW:
                    mm.ins.ldweights = False
            vv = v_view(m, h)
            for i in range(2):
                mm = nc.tensor.matmul(pv_t[(h, i)], vv, at[:, i, :],
                                      start=(m == 0), stop=(m == NM - 1))
                if i == 1 and SHARE_LDW:
                    mm.ins.ldweights = False

        from concourse.dve_ops import (
            RECIP_APPROX_FAST_CONSTS,
            RECIPROCAL_APPROX_FAST,
        )

        def recip_bcast(h):
            rs = rs_t.pop(h)
            cc = RECIP_APPROX_FAST_CONSTS
            # one recip over partitions 0..32 (base-0 AP: the custom DVE op
            # mis-reads non-zero partition offsets); rows 1..31 are unused
            rc = RC.tile([33, HALF], F32, tag="rc", name=f"rc{h}")
            nc.vector._custom_dve(RECIPROCAL_APPROX_FAST, out=rc,
                                  in0=rs[0:33, :], s0=cc["s0"], s1=cc["s1"],
                                  imm2=cc["imm2"])
            # half 1's row must be re-staged at partition base 0 for gpsimd
            rc1 = RC.tile([1, HALF], F32, tag="rc1", name=f"rc1_{h}")
            nc.vector.tensor_copy(rc1, rc[32:33, :])
            for i, src_ap in ((0, rc[0:1, :]), (1, rc1)):
                bc = BC.tile([128, HALF], F32, tag="bc", name=f"bc{h}_{i}")
                nc.gpsimd.partition_broadcast(bc, src_ap, channels=128)
                bc_t[(h, i)] = bc
            rc_t[h] = rc

        def oh_mul(h, i):
            oh = OH.tile([128, HALF], BF16, tag="oh", name=f"oh{h}_{i}")
            nc.vector.tensor_mul(oh, pv_t.pop((h, i)), bc_t.pop((h, i)))
            rc_t.pop(h, None)
            oh_t[(h, i)] = oh

        def proj_acc(h, i):
            ns = slice(i * HALF, (i + 1) * HALF)
            pj = PS_PJ.tile([128, HALF], F32, tag="pj", name=f"pj{h}_{i}")
            nc.tensor.matmul(pj, pw_view(h), oh_t.pop((h, i)),
                             start=True, stop=True)
            if h == 0:
                nc.vector.tensor_copy(yacc[:, ns], pj)
            else:
                nc.vector.tensor_add(yacc[:, ns], yacc[:, ns], pj)

        def finalize(i):
            ns = slice(i * HALF, (i + 1) * HALF)
            nc.scalar.activation(yt_sb[:, ns], yacc[:, ns], func=IDENT,
                                 bias=pb_sb)
            nc.sync.dma_start(out=yT[:, ns], in_=yt_sb[:, ns])

        pi = 0
        for u in range(H + 2):
            quota = piece_quota.get(u, 0)
            for m in range(NM):
                if m == 0 and u >= 2:
                    oh_mul(u - 2, 0)   # frees pv buffers before reuse
                    oh_mul(u - 2, 1)
                if u < H:
                    s_exp(u, m)
                if 1 <= u <= H:
                    ones_pv(u - 1, m)
                if m == 2 and u >= 2:
                    proj_acc(u - 2, 0)
                if m == 5 and u >= 2:
                    proj_acc(u - 2, 1)
                if quota:
                    for _ in range(quota // NM + (1 if m < quota % NM else 0)):
                        pieces[pi](); pi += 1
            if 1 <= u <= H:
                recip_bcast(u - 1)   # rs(u-1) just stopped
            if u == H + 1:
                finalize(0)
                finalize(1)
        assert pi == len(pieces), (pi, len(pieces))


_CACHE = {}


def _prep_inputs(x, B_bias, wq_w, wq_b, wk_w, wk_b, wv_w, wv_b, proj_w, proj_b):
    s = 1.0 / math.sqrt(DH)
    f = np.float32
    b16 = ml_dtypes.bfloat16

    def d2(w):  # [256, C] -> [2, 128, C]
        return np.asarray(w, f).reshape(2, 128, -1)

    wq3 = d2(np.asarray(wq_w) * s)
    wk3 = d2(wk_w)
    wv3 = d2(wv_w)
    # wqk0: [p, w, a, c0] packed
    wqk0 = np.stack([wq3[:, :, :128], wk3[:, :, :128]], 0)  # [w, a, p, 128]
    wqk0 = np.ascontiguousarray(
        wqk0.transpose(2, 0, 1, 3).reshape(128, -1).astype(b16))
    # wbig per d: [wq_r 896 | wk_r 896 | pw-half 512]
    pwf = np.asarray(proj_w, f).reshape(2, 512, DH)  # head-halves 0-3 / 4-7
    rows = []
    for d in range(2):
        pw_tail = pwf[d].reshape(4, 128, DH)
        pw_part = pw_tail.transpose(1, 0, 2).reshape(128, 512)
        rows.append(np.concatenate(
            [wq3[d, :, 128:], wk3[d, :, 128:], pw_part], axis=1))
    wbig = np.ascontiguousarray(np.stack(rows, 1).reshape(128, -1).astype(b16))
    wv_p = np.ascontiguousarray(
        wv3.transpose(1, 0, 2).reshape(128, -1).astype(b16))
    # biases: [wqb 8 | pb2 1]; k-bias dropped (cancels in softmax),
    # v-bias folded into the projection bias.
    wqb_t = (np.asarray(wq_b, f) * s).reshape(8, 128).T
    pb2 = (np.asarray(proj_b, f)
           + np.asarray(wv_b, f) @ np.asarray(proj_w, f)).reshape(128, 1)
    bias_all = np.ascontiguousarray(
        np.concatenate([wqb_t, pb2], axis=1).astype(f))
    # eb: exp(B)^T chunks packed [p, (m n)]
    ebh = np.exp(np.asarray(B_bias, np.float32).T).reshape(8, 128, N)
    eb_all = np.ascontiguousarray(
        ebh.transpose(1, 0, 2).reshape(128, 8 * N).astype(b16))
    xTh = np.asarray(x, f).transpose(0, 2, 1).reshape(8, 2, 128, N)
    shared = dict(wqk0=wqk0, wbig=wbig, wv=wv_p, biases=bias_all, eb=eb_all)
    return [dict(shared, xt=np.ascontiguousarray(
        xTh[b].transpose(1, 0, 2).reshape(128, 2 * N).astype(b16)))
        for b in range(NCORES)]


def kernel(**inputs):
    from concourse.bass_utils import run_bass_kernel_spmd

    if "nc" not in _CACHE:
        _CACHE["nc"] = build_nc()
    nc = _CACHE["nc"]
    in_maps = _prep_inputs(**inputs)
    res = run_bass_kernel_spmd(nc, in_maps, core_ids=list(range(NCORES)))
    out = np.stack([np.asarray(res.results[b]["yT"]).T for b in range(NCORES)])
    return np.ascontiguousarray(out.astype(np.float32))


# revision 21
# speedup vs baseline: 1.1769x; 1.0081x over previous
"""Multi-head attention block (B=8, N=1024, H=8, d=128, D_in=256) on 8 trn2 cores.

Sharding: data-parallel over batch — core b computes batch element b entirely
(8 heads), no collectives. Host pre-transposes x, pre-scales wq by 1/sqrt(d),
and ships the additive [N,N] bias as exp(B)^T so the device computes
exp(S+B) = exp(S) * expB.

bf16 pipeline, HEAD-granular pipeline with stationary-weight sharing:
  * TRN2 serializes each matmul behind its ~90ns stationary load; a matmul
    with `ldweights=False` reuses the previously loaded stationary
    (measured -73ns/matmul). The loop processes both n-halves of a head
    together so S (shared KT chunk), PV (shared V slice) and the QKV
    setup matmuls (shared weight chunk) run as no-reload pairs.
  * S matmuls for the two n-halves of m-chunk m land in one 2-bank PSUM
    group [128, 2(half), 512]; ONE ACT exp per m (full row), ONE DVE mul
    by the exp(B)^T row (bf16 16-bit 2x mode).
  * softmax denominators: per-m ones matmuls accumulate into a shared
    PSUM bank (half 0 at partition 0, half 1 at partition 32); DVE
    approx-reciprocal per half; GPSIMD partition_broadcast expands
    rc[1,512] -> bc[128,512] SBUF (no PE broadcast matmul, no ACT copy).
  * K-bias dropped (cancels in softmax); V-bias folded into proj_b on the
    host; Q-bias rides the ACT PSUM->SBUF copy.
  (fp8 DoubleRow was tried and reverted: real-TRN2 DR matmuls stream at
  the same rows/cycle as bf16, unlike the cost model.)

PSUM banks: S pool 2 groups x 2 banks (also serves setup pieces), pv 2,
rs 1 (both halves), pj 1.

Per-core dataflow (all matmuls bf16, moving free dim 512):
  QT[c][128,2,512], KT, V via setup pieces (4 matmuls + 1 chunk copy each)
  head loop u (software-pipelined 2 deep), m-chunk loop 0..7:
    S pair (shared KT stationary) -> exp -> at[128,2,512] *= expB row
    rs pair (ones) + pv pair (shared V stationary) consume at[:, i, :]
  2x recip + gpsimd bcast per head; oh = pv * bc; pj = pw_h.T @ oh;
  yacc += pj;  yT = yacc + proj_b' -> DRAM [128, 1024]; host transposes.
"""

import math
import sys

import numpy as np

if "/opt/trn_rl_repo" not in sys.path:
    sys.path.insert(0, "/opt/trn_rl_repo")

import ml_dtypes

import concourse.bass as bass
import concourse.bass_isa as bass_isa
import concourse.tile as tile
from concourse import bacc
from concourse import library_config
from concourse import mybir

F32 = mybir.dt.float32
BF16 = mybir.dt.bfloat16
EXP = mybir.ActivationFunctionType.Exp
IDENT = mybir.ActivationFunctionType.Identity

N = 1024          # sequence length
D_IN = 256        # input dim
H = 8             # heads
DH = 128          # head dim
C = H * DH        # 1024
NCORES = 8
HALF = 512        # matmul moving free dim
NM = 8            # m-chunks per head
SHARE_LDW = True  # reuse loaded stationaries (ldweights=False on 2nd of pair)


def build_nc():
    nc = bacc.Bacc("TRN2", target_bir_lowering=False, debug=False,
                   num_devices=NCORES)

    xt_d = nc.dram_tensor("xt", [128, 2 * N], BF16, kind="ExternalInput").ap()
    wqk0_d = nc.dram_tensor("wqk0", [128, 4 * 128], BF16,
                            kind="ExternalInput").ap()
    bias_d = nc.dram_tensor("biases", [128, 9], F32,
                            kind="ExternalInput").ap()
    wv_d = nc.dram_tensor("wv", [128, 2 * 1024], BF16,
                          kind="ExternalInput").ap()
    wbig_d = nc.dram_tensor("wbig", [128, 2 * 2304], BF16,
                            kind="ExternalInput").ap()
    eb_d = nc.dram_tensor("eb", [128, 8 * N], BF16, kind="ExternalInput").ap()
    yT = nc.dram_tensor("yT", [DH, N], F32, kind="ExternalOutput").ap()

    with tile.TileContext(nc) as tc:
        build_body(nc, tc, xt_d, wqk0_d, bias_d, wv_d, wbig_d, eb_d, yT)
    nc.compile()
    return nc


def build_body(nc, tc, xt_d, wqk0_d, bias_d, wv_d, wbig_d, eb_d, yT):
    with (
        tc.tile_pool(name="persist", bufs=1) as P,
        tc.tile_pool(name="at", bufs=10) as AT,
        tc.tile_pool(name="oh", bufs=3) as OH,
        tc.tile_pool(name="rc", bufs=3) as RC,
        tc.tile_pool(name="bc", bufs=3) as BC,
        tc.tile_pool(name="ps_s", bufs=2, space="PSUM") as PS_S,
        tc.tile_pool(name="ps_pv", bufs=2, space="PSUM") as PS_PV,
        tc.tile_pool(name="ps_rs", bufs=1, space="PSUM") as PS_RS,
        tc.tile_pool(name="ps_pj", bufs=1, space="PSUM") as PS_PJ,
    ):
        # gpsimd library for partition_broadcast; loads while inputs DMA
        nc.gpsimd.load_library(library_config.attn)

        # ---- input DMAs: critical-path first (wqk0 is tiny; the first
        # setup matmul needs it plus xt chunk 0 only)
        wqk0 = P.tile([128, 2, 2, 128], BF16, tag="wqk0")
        nc.sync.dma_start(out=wqk0, in_=wqk0_d.rearrange(
            "p (w a c) -> p w a c", w=2, a=2))
        xt_all = [P.tile([128, N], BF16, tag=f"xt{d}", name=f"xt{d}")
                  for d in range(2)]
        for d in range(2):
            nc.sync.dma_start(out=xt_all[d], in_=xt_d[:, d * N:(d + 1) * N])
        bias_all = P.tile([128, 9], F32, tag="bias")
        nc.sync.dma_start(out=bias_all, in_=bias_d)
        wv_sb = P.tile([128, 2, 1024], BF16, tag="wv")
        nc.sync.dma_start(out=wv_sb, in_=wv_d.rearrange("p (a c) -> p a c",
                                                        a=2))
        wbig = P.tile([128, 2, 2304], BF16, tag="wbig")
        nc.sync.dma_start(out=wbig, in_=wbig_d.rearrange("p (a c) -> p a c",
                                                         a=2))
        # eb rows as [128, 4, 2, 512] so a full n-row views as [128, 2, 512]
        eb_sb = [P.tile([128, 4, 2, HALF], BF16, tag=f"eb{h}", name=f"eb{h}")
                 for h in range(2)]
        for h in range(2):
            nc.sync.dma_start(out=eb_sb[h], in_=eb_d[:, h * 4 * N:(h + 1) * 4 * N]
                              .rearrange("p (a i n) -> p a i n", a=4, i=2))

        def eb_row(m):   # [128, 2, 512] view of expB for m-chunk m
            return eb_sb[m // 4][:, m % 4, :, :]

        wqb_sb = bias_all[:, 0:8]
        pb_sb = bias_all[:, 8:9]

        def pw_view(h):  # pw head h lives in the d=h//4 tail of wbig
            o = 1792 + (h % 4) * 128
            return wbig[:, h // 4, o:o + 128]

        # ---- persistent tiles ----
        ones = P.tile([128, 1], BF16, tag="ones")
        with tc.tile_pool(name="mkconst", bufs=1) as MK:
            ones_f = MK.tile([128, 1], F32, tag="ones_f")
            nc.vector.memset(ones_f, 1.0)
            nc.vector.tensor_copy(ones, ones_f)
            warm = MK.tile([128, 1], F32, tag="warm")
            nc.scalar.activation(warm, ones_f, func=EXP)
        qt_sb = [P.tile([128, 2, HALF], BF16, tag=f"qt{c}", name=f"qt{c}")
                 for c in range(8)]
        kt_sb = [P.tile([128, 2, HALF], BF16, tag=f"kt{c}", name=f"kt{c}")
                 for c in range(8)]
        v_sb = [P.tile([128, 2, HALF], BF16, tag=f"v{n}", name=f"v{n}")
                for n in range(8)]

        def kt_view(h, m):   # kt head h, m-chunk slice [128, 128]
            return kt_sb[h][:, m // 4, (m % 4) * 128:(m % 4 + 1) * 128]

        def v_view(m, h):    # v m-chunk, head-dim slice [128, 128]
            return v_sb[m][:, h // 4, (h % 4) * 128:(h % 4 + 1) * 128]

        yacc = P.tile([128, N], F32, tag="yacc")
        yt_sb = P.tile([128, N], F32, tag="yt")

        # ---- setup pieces: d-major matmul order shares each stationary
        def qkt_piece(wname, dst, c):
            wi = 0 if wname == "wq" else 1
            g = PS_S.tile([128, 2, HALF], F32, tag="sg")
            for d in range(2):
                if c == 0:
                    wt = wqk0[:, wi, d, :]
                else:
                    wt = wbig[:, d, wi * 896 + (c - 1) * 128:
                              wi * 896 + c * 128]
                for i in range(2):
                    mm = nc.tensor.matmul(
                        g[:, i, :], wt, xt_all[d][:, i * HALF:(i + 1) * HALF],
                        start=(d == 0), stop=(d == 1))
                    if i == 1 and SHARE_LDW:
                        mm.ins.ldweights = False
            if wname == "wq":
                nc.scalar.activation(dst[c], g, func=IDENT,
                                     bias=wqb_sb[:, c:c + 1])
            else:
                nc.vector.tensor_copy(dst[c], g)

        def v_piece(n):
            nsl = slice(n * 128, (n + 1) * 128)
            g = PS_S.tile([128, 2, HALF], F32, tag="sg")
            for d in range(2):
                for i in range(2):
                    mm = nc.tensor.matmul(
                        g[:, i, :], xt_all[d][:, nsl],
                        wv_sb[:, d, i * HALF:(i + 1) * HALF],
                        start=(d == 0), stop=(d == 1))
                    if i == 1 and SHARE_LDW:
                        mm.ins.ldweights = False
            nc.vector.tensor_copy(v_sb[n], g)

        qkt_piece("wq", qt_sb, 0)
        qkt_piece("wk", kt_sb, 0)

        pieces = [lambda n=n: v_piece(n) for n in range(8)]
        for c in range(1, 8):
            pieces.append(lambda c=c: qkt_piece("wq", qt_sb, c))
            pieces.append(lambda c=c: qkt_piece("wk", kt_sb, c))
        # chunk c needed at head-iteration c; v needed at u=1
        piece_quota = {0: 10, 1: 2, 2: 2, 3: 2, 4: 2, 5: 2, 6: 2}

        # ---- pipelined head loop ----
        at_t = {}     # (h, m) -> at tile [128, 2(half), 512]
        pv_t = {}     # (h, i) -> pv psum tile
        rs_t = {}     # h -> rowsum psum bank (half 0 @ p0, half 1 @ p32)
        rc_t = {}     # (h, i) -> reciprocal rowsum [1, HALF]
        bc_t = {}     # (h, i) -> broadcast recip [128, HALF] SBUF
        oh_t = {}     # (h, i) -> normalized head-output tile

        def s_exp(h, m):
            g = PS_S.tile([128, 2, HALF], F32, tag="sg", name=f"sg{h}_{m}")
            kt = kt_view(h, m)
            for i in range(2):
                mm = nc.tensor.matmul(g[:, i, :], kt, qt_sb[h][:, i, :],
                                      start=True, stop=True)
                if i == 1 and SHARE_LDW:
                    mm.ins.ldweights = False
            at = AT.tile([128, 2, HALF], BF16, tag="at", name=f"at{h}_{m}")
            nc.scalar.activation(at, g, func=EXP)
            nc.vector.tensor_mul(at, at, eb_row(m))
            at_t[(h, m)] = at

        def ones_pv(h, m):
            if m == 0:
                rs_t[h] = PS_RS.tile([128, HALF], F32, tag="rs", name=f"rs{h}")
                for i in range(2):
                    pv_t[(h, i)] = PS_PV.tile([128, HALF], F32, tag="pv",
                                              name=f"pv{h}_{i}")
            at = at_t.pop((h, m))
            rs = rs_t[h]
            for i in range(2):
                mm = nc.tensor.matmul(rs[32 * i:32 * i + 1, :], ones,
                                      at[:, i, :], start=(m == 0),
                                      stop=(m == NM - 1))
                if i == 1 and SHARE_LDW:
                    mm.ins.ldweights = False
            vv = v_view(m, h)
            for i in range(2):
                mm = nc.tensor.matmul(pv_t[(h, i)], vv, at[:, i, :],
                                      start=(m == 0), stop=(m == NM - 1))
                if i == 1 and SHARE_LDW:
                    mm.ins.ldweights = False

        from concourse.dve_ops import (
            RECIP_APPROX_FAST_CONSTS,
            RECIPROCAL_APPROX_FAST,
        )

        def recip_bcast(h):
            rs = rs_t.pop(h)
            cc = RECIP_APPROX_FAST_CONSTS
            # one recip over partitions 0..32 (base-0 AP: the custom DVE op
            # mis-reads non-zero partition offsets); rows 1..31 are unused
            rc = RC.tile([33, HALF], F32, tag="rc", name=f"rc{h}")
            nc.vector._custom_dve(RECIPROCAL_APPROX_FAST, out=rc,
                                  in0=rs[0:33, :], s0=cc["s0"], s1=cc["s1"],
                                  imm2=cc["imm2"])
            # half 1's row must be re-staged at partition base 0 for gpsimd
            rc1 = RC.tile([1, HALF], F32, tag="rc1", name=f"rc1_{h}")
            nc.vector.tensor_copy(rc1, rc[32:33, :])
            for i, src_ap in ((0, rc[0:1, :]), (1, rc1)):
                bc = BC.tile([128, HALF], F32, tag="bc", name=f"bc{h}_{i}")
                nc.gpsimd.partition_broadcast(bc, src_ap, channels=128)
                bc_t[(h, i)] = bc
            rc_t[h] = rc

        def oh_mul(h, i):
            oh = OH.tile([128, HALF], BF16, tag="oh", name=f"oh{h}_{i}")
            nc.vector.tensor_mul(oh, pv_t.pop((h, i)), bc_t.pop((h, i)))
            rc_t.pop(h, None)
            oh_t[(h, i)] = oh

        def proj_acc(h, i):
            ns = slice(i * HALF, (i + 1) * HALF)
            pj = PS_PJ.tile([128, HALF], F32, tag="pj", name=f"pj{h}_{i}")
            nc.tensor.matmul(pj, pw_view(h), oh_t.pop((h, i)),
                             start=True, stop=True)
            if h == 0:
                nc.vector.tensor_copy(yacc[:, ns], pj)
            else:
                nc.vector.tensor_add(yacc[:, ns], yacc[:, ns], pj)

        def finalize(i):
            ns = slice(i * HALF, (i + 1) * HALF)
            nc.scalar.activation(yt_sb[:, ns], yacc[:, ns], func=IDENT,
                                 bias=pb_sb)
            nc.sync.dma_start(out=yT[:, ns], in_=yt_sb[:, ns])

        pi = 0
        for u in range(H + 2):
            quota = piece_quota.get(u, 0)
            for m in range(NM):
                if m == 0 and u >= 2:
                    oh_mul(u - 2, 0)   # frees pv buffers before reuse
                    oh_mul(u - 2, 1)
                if u < H:
                    s_exp(u, m)
                if 1 <= u <= H:
                    ones_pv(u - 1, m)
                if m == 2 and u >= 2:
                    proj_acc(u - 2, 0)
                    if u == H + 1:
                        finalize(0)
                if m == 5 and u >= 2:
                    proj_acc(u - 2, 1)
                    if u == H + 1:
                        finalize(1)
                if quota:
                    for _ in range(quota // NM + (1 if m < quota % NM else 0)):
                        pieces[pi](); pi += 1
            if 1 <= u <= H:
                recip_bcast(u - 1)   # rs(u-1) just stopped

        assert pi == len(pieces), (pi, len(pieces))


_CACHE = {}


def _prep_inputs(x, B_bias, wq_w, wq_b, wk_w, wk_b, wv_w, wv_b, proj_w, proj_b):
    s = 1.0 / math.sqrt(DH)
    f = np.float32
    b16 = ml_dtypes.bfloat16

    def d2(w):  # [256, C] -> [2, 128, C]
        return np.asarray(w, f).reshape(2, 128, -1)

    wq3 = d2(np.asarray(wq_w) * s)
    wk3 = d2(wk_w)
    wv3 = d2(wv_w)
    # wqk0: [p, w, a, c0] packed
    wqk0 = np.stack([wq3[:, :, :128], wk3[:, :, :128]], 0)  # [w, a, p, 128]
    wqk0 = np.ascontiguousarray(
        wqk0.transpose(2, 0, 1, 3).reshape(128, -1).astype(b16))
    # wbig per d: [wq_r 896 | wk_r 896 | pw-half 512]
    pwf = np.asarray(proj_w, f).reshape(2, 512, DH)  # head-halves 0-3 / 4-7
    rows = []
    for d in range(2):
        pw_tail = pwf[d].reshape(4, 128, DH)
        pw_part = pw_tail.transpose(1, 0, 2).reshape(128, 512)
        rows.append(np.concatenate(
            [wq3[d, :, 128:], wk3[d, :, 128:], pw_part], axis=1))
    wbig = np.ascontiguousarray(np.stack(rows, 1).reshape(128, -1).astype(b16))
    wv_p = np.ascontiguousarray(
        wv3.transpose(1, 0, 2).reshape(128, -1).astype(b16))
    # biases: [wqb 8 | pb2 1]; k-bias dropped (cancels in softmax),
    # v-bias folded into the projection bias.
    wqb_t = (np.asarray(wq_b, f) * s).reshape(8, 128).T
    pb2 = (np.asarray(proj_b, f)
           + np.asarray(wv_b, f) @ np.asarray(proj_w, f)).reshape(128, 1)
    bias_all = np.ascontiguousarray(
        np.concatenate([wqb_t, pb2], axis=1).astype(f))
    # eb: exp(B)^T chunks packed [p, (m n)]
    ebh = np.exp(np.asarray(B_bias, np.float32).T).reshape(8, 128, N)
    eb_all = np.ascontiguousarray(
        ebh.transpose(1, 0, 2).reshape(128, 8 * N).astype(b16))
    xTh = np.asarray(x, f).transpose(0, 2, 1).reshape(8, 2, 128, N)
    shared = dict(wqk0=wqk0, wbig=wbig, wv=wv_p, biases=bias_all, eb=eb_all)
    return [dict(shared, xt=np.ascontiguousarray(
        xTh[b].transpose(1, 0, 2).reshape(128, 2 * N).astype(b16)))
        for b in range(NCORES)]


def kernel(**inputs):
    from concourse.bass_utils import run_bass_kernel_spmd

    if "nc" not in _CACHE:
        _CACHE["nc"] = build_nc()
    nc = _CACHE["nc"]
    in_maps = _prep_inputs(**inputs)
    res = run_bass_kernel_spmd(nc, in_maps, core_ids=list(range(NCORES)))
    out = np.stack([np.asarray(res.results[b]["yT"]).T for b in range(NCORES)])
    return np.ascontiguousarray(out.astype(np.float32))
